# revision 31
# baseline (speedup 1.0000x reference)
"""Conv2d(128->256, 3x3, pad 1, stride 1) on 32x56x56 fp32, for 8 trn2 cores.

Strategy: data-parallel over batch N=32 -> 4 images/core. Per core an
implicit-GEMM conv: C_in=128 is the partition (contraction) dim; for each
(kh, kw) tap a [128ci x 128co] weight tile multiplies a shifted window of the
column-padded input image held in SBUF, accumulating into PSUM over the 9
taps. Output rows are processed in chunks of 8 (free 8*56=448 <= 512 PSUM
bank). Matmuls run in float16 (~4.5e-4 rel err) with fp32 PSUM accumulate.

Tuned from perfetto traces:
- HOST-PREPADDED input: the host builds, per image, a [128, 58*116] fp16
  block of interleaved row pairs [A-row | B-row]: A has the interior at
  column 1 with zero border columns (serves kw=0/2 taps at even fp16
  offsets), B at column 2 (serves kw=1; odd rhs offsets cost ~18 PE
  cycles/matmul in SBUF word-split reads). One DMA per image lands
  matmul-ready data: no on-device pad copies or memsets at all, so the
  DVE/ACT engines can never head-of-line-block the PE's dependencies.
- No zero pad ROWS: boundary chunks shrink the kh taps that would read them
  (448 -> 392 free, PSUM sub-range), saving ~1.1us of PE work.
- PE warmup: the PE clock ramps to full only after ~3-6us of CONTINUOUS
  execution and re-gates after ~1us idle. Dummy matmuls keep it busy from
  the earliest instruction a queue can run: first const-AP dummies (no
  runtime memset dependency), then wu-based ones, shrinking toward the end
  for a fine-grained handoff. The runway is overprovisioned: a late-input
  core pays ~1.5us for a PE idle, an excess dummy only ~45ns.
- All four images' input DMAs are issued dependency-free at kernel start:
  ring-FIFO order guarantees image-0's slices drain first, and all input
  traffic completes before the first bulk output DMA (whose 6-12KB
  descriptors occupy a queue ~0.25-0.5us each) can starve anything.
- bias-add (PSUM->SBUF, f32->f16) on the ACT engine via activation-Identity.
- Outputs fp16 (host upcasts; halves DMA bytes). Bulk per-half DMAs for
  images 0-2; the last image streams chunk-pairs out as produced on
  alternating rings, ending with 6-row and 2-row chunks so the
  post-final-matmul ADD+DMA tail is tiny.
"""
import numpy as np
from contextlib import ExitStack

N_FULL, C_IN, H, W = 32, 128, 56, 56
C_OUT, KS = 256, 3
N_CORES = 8
N_PER = N_FULL // N_CORES          # 4 images per core
PIX = H * W                         # 3136
ROWS = 8                            # output rows per psum chunk
RC = H // ROWS                      # 7 chunks
NF = ROWS * W                       # 448 free elems per matmul
SW = 58                             # padded row width (56 + 2 border cols)
SW2 = 2 * SW                        # interleaved A|B row pair width

T_ROWS = 33                         # top region: x rows 0..32 (chunks 0-3)
B_ROWS = 25                         # bottom region: x rows 31..55 (chunks 4-6)
B0 = 31                             # first x row in the bottom region
BLK = (T_ROWS + B_ROWS) * SW2       # 6728 elems per partition per image
XT_A = 9                            # image-0 sub-DMA 1: x rows 0..8 (chunk 0)

# tap order per chunk kind: (kh, kw) tuples. start=first, stop=last.
# narrow taps (boundary rows) must not be first or last.
TAPS_MID = [(0, 0), (0, 1), (0, 2), (1, 0), (1, 1), (1, 2), (2, 0), (2, 1), (2, 2)]
TAPS_TOP = [(1, 0), (1, 1), (1, 2), (0, 0), (0, 1), (0, 2), (2, 0), (2, 1), (2, 2)]
TAPS_BOT = [(0, 0), (0, 1), (0, 2), (2, 0), (2, 1), (2, 2), (1, 0), (1, 1), (1, 2)]

_CACHE = {}


def _build():
    import concourse.tile as tile
    from concourse import mybir, bacc

    f32 = mybir.dt.float32
    f16 = mybir.dt.float16

    nc = bacc.Bacc("TRN2", target_bir_lowering=False, debug=False)
    # host-prepadded interleaved A|B blocks, one per image (see module doc)
    x_d = nc.dram_tensor("x", [N_PER, C_IN, BLK], f16, kind="ExternalInput").ap()
    # host-pretransposed: [ci, half, k, co_half] (half-major, contiguous per
    # half); within each half taps are stored in TAPS_TOP order so chunk-0's
    # first weights DMA first.
    w_d = nc.dram_tensor("w", [C_IN, 2, KS * KS, 128], f16, kind="ExternalInput").ap()
    b_d = nc.dram_tensor("b", [C_OUT], f32, kind="ExternalInput").ap()
    # fp16 output (upcast on host): halves output DMA bytes; ~5e-4 rel err
    y_d = nc.dram_tensor("y", [N_PER, C_OUT, PIX], f16, kind="ExternalOutput").ap()

    # host-order index of tap (kh, kw) inside a weight half (TAPS_TOP order)
    widx = {t: i for i, t in enumerate(TAPS_TOP)}

    with tile.TileContext(nc) as tc:
        with ExitStack() as ctx:
            wp = ctx.enter_context(tc.tile_pool(name="wp", bufs=1))
            xblk = ctx.enter_context(tc.tile_pool(name="xblk", bufs=3))
            pp = ctx.enter_context(tc.tile_pool(name="pp", bufs=6, space="PSUM"))
            op = ctx.enter_context(tc.tile_pool(name="op", bufs=2))

            # PE warmup runway (see module doc)
            c0 = nc.const_aps.tensor(0.0, [128, 1], f32)
            wups = pp.tile([128, NF], f32, tag="ps")
            for _ in range(10):
                nc.tensor.matmul(wups[0:1, 0:1], c0, c0, start=True, stop=True)
            wu = wp.tile([128, 224], f16)
            nc.gpsimd.memset(wu[:], 0.0)
            for _ in range(8):
                nc.tensor.matmul(wups[:, 0:224], wu[:, 0:128], wu[:], start=True, stop=True)
            for _ in range(24):
                nc.tensor.matmul(wups[:, 0:112], wu[:, 0:128], wu[:, 0:112], start=True, stop=True)

            # Weight half 0 in two pieces on the ACT ring: chunk 0's first six
            # taps' weights (192KB) gate the first matmuls; the rest follows.
            w_r = wp.tile([C_IN, 2 * KS * KS * 128], f16)
            w_r4 = w_r[:].rearrange("p (h k co) -> p h k co", h=2, k=KS * KS)
            nc.scalar.dma_start(
                w_r4[:, 0, 0:6], w_d[:, 0, 0:6].rearrange("ci k co -> ci (k co)")
            )
            nc.scalar.dma_start(
                w_r4[:, 0, 6:9], w_d[:, 0, 6:9].rearrange("ci k co -> ci (k co)")
            )

            bias_sb = wp.tile([128, 2], f32)

            for n in range(N_PER):
                xb = xblk.tile([C_IN, BLK], f16)
                if n == 0:
                    # image 0 in three slices so chunk 0 unblocks early
                    nc.sync.dma_start(xb[:, 0 : XT_A * SW2], x_d[n, :, 0 : XT_A * SW2])
                    nc.sync.dma_start(
                        xb[:, XT_A * SW2 : T_ROWS * SW2],
                        x_d[n, :, XT_A * SW2 : T_ROWS * SW2],
                    )
                    nc.sync.dma_start(
                        xb[:, T_ROWS * SW2 :], x_d[n, :, T_ROWS * SW2 :]
                    )
                    # weight half 1 + bias queue behind the critical pieces
                    nc.scalar.dma_start(
                        w_r4[:, 1], w_d[:, 1].rearrange("ci k co -> ci (k co)")
                    )
                    nc.scalar.dma_start(bias_sb[:], b_d.rearrange("(h p) -> p h", h=2))
                else:
                    nc.sync.dma_start(xb[:], x_d[n])

                top3 = xb[:, 0 : T_ROWS * SW2].rearrange("p (a b) -> p a b", a=T_ROWS)
                bot3 = xb[:, T_ROWS * SW2 :].rearrange("p (a b) -> p a b", a=B_ROWS)

                out_sb = op.tile([128, 2 * PIX], f16)
                last_img = n == N_PER - 1
                for half in range(2):
                    # the very last half ends with 6-row + 2-row chunks so the
                    # post-final-matmul ADD+DMA tail is tiny
                    if last_img and half == 1:
                        chunks = [(i * ROWS, ROWS) for i in range(6)] + [(48, 6), (54, 2)]
                    else:
                        chunks = [(i * ROWS, ROWS) for i in range(RC)]
                    for ci, (r0c, nr) in enumerate(chunks):
                        nfc = nr * W
                        ps = pp.tile([128, NF], f32)
                        top_chunk = r0c == 0
                        bot_chunk = r0c + nr == H
                        taps = TAPS_TOP if top_chunk else (TAPS_BOT if bot_chunk else TAPS_MID)
                        for i, (kh, kw) in enumerate(taps):
                            lhsT = w_r4[:, half, widx[(kh, kw)], :]
                            # output rows r0c..r0c+nr-1 read x rows r0c+kh-1..
                            r0 = r0c + kh - 1
                            narrow_top = top_chunk and kh == 0     # skip out row 0
                            narrow_bot = bot_chunk and kh == 2     # skip out row 55
                            src3, base = (top3, 0) if r0c < 32 else (bot3, B0)
                            # A sub-row at col 0, B sub-row at col 58 of each
                            # 116-wide pair; interior at 1 (A) / 60 (B)
                            coff = 58 + 2 if kw == 1 else kw
                            if narrow_top:
                                rhs = src3[:, 0 : nr - 1, coff : coff + W]
                                dst = ps[:, W:nfc]
                            elif narrow_bot:
                                lr = r0 - base
                                rhs = src3[:, lr : lr + nr - 1, coff : coff + W]
                                dst = ps[:, 0 : nfc - W]
                            else:
                                lr = r0 - base
                                rhs = src3[:, lr : lr + nr, coff : coff + W]
                                dst = ps[:, 0:nfc]
                            nc.tensor.matmul(
                                dst, lhsT, rhs,
                                start=(i == 0), stop=(i == KS * KS - 1),
                            )
                        # psum -> sbuf with per-channel bias add (f32 -> f16)
                        # on the ACT engine: keeps it off the PE critical path
                        lo = half * PIX + r0c * W
                        nc.scalar.activation(
                            out_sb[:, lo : lo + nfc],
                            ps[:, 0:nfc],
                            mybir.ActivationFunctionType.Identity,
                            bias=bias_sb[:, half : half + 1],
                        )
                        if last_img:
                            # last image: stream output out as produced so
                            # nothing bulky queues ahead of the tail. Pair up
                            # 8-row chunks (fewer DMAs -> fewer semaphores to
                            # clear in teardown); alternate rings so
                            # consecutive writes drain in parallel.
                            paired = (
                                nr == ROWS and ci % 2 == 0
                                and ci + 1 < len(chunks) and chunks[ci + 1][1] == ROWS
                            )
                            if paired:
                                pass  # written together with the next chunk
                            else:
                                w0 = (r0c - ROWS) * W if (nr == ROWS and ci % 2 == 1) else r0c * W
                                eng = nc.sync if (half == 0) == (ci % 2 == 0) else nc.gpsimd
                                eng.dma_start(
                                    y_d[n, half * 128 : (half + 1) * 128, w0 : r0c * W + nfc],
                                    out_sb[:, half * PIX + w0 : lo + nfc],
                                )
                    if not last_img:
                        eng = nc.scalar if half == 0 else nc.gpsimd
                        eng.dma_start(
                            y_d[n, half * 128 : (half + 1) * 128, :],
                            out_sb[:, half * PIX : (half + 1) * PIX],
                        )
    nc.compile()
    return nc


def _get_nc():
    if "nc" not in _CACHE:
        _CACHE["nc"] = _build()
    return _CACHE["nc"]


def _prep_inputs(x, weight, bias):
    x = np.asarray(x, dtype=np.float32).astype(np.float16)  # [N, C, 56, 56]
    # interleaved prepadded block per image: rows r=0..57 map to x rows
    # [0..32] (top) then [31..55] (bottom); each row is [A(58) | B(58)]:
    # A = [0, x_r, 0], B = [0, 0, x_r]
    blk = np.zeros((N_FULL, C_IN, T_ROWS + B_ROWS, SW2), dtype=np.float16)
    blk[:, :, 0:T_ROWS, 1 : 1 + W] = x[:, :, 0:T_ROWS, :]
    blk[:, :, 0:T_ROWS, SW + 2 : SW + 2 + W] = x[:, :, 0:T_ROWS, :]
    blk[:, :, T_ROWS:, 1 : 1 + W] = x[:, :, B0 : B0 + B_ROWS, :]
    blk[:, :, T_ROWS:, SW + 2 : SW + 2 + W] = x[:, :, B0 : B0 + B_ROWS, :]
    blk = np.ascontiguousarray(blk.reshape(N_FULL, C_IN, BLK))
    # [co, ci, kh, kw] -> [ci, half, k, co_half], half-major; taps within a
    # half stored in TAPS_TOP order so chunk-0's first weights DMA first.
    w4 = (
        np.transpose(np.asarray(weight, dtype=np.float32), (1, 2, 3, 0))
        .reshape(C_IN, KS * KS, 2, 128)
        .transpose(0, 2, 1, 3)
    )  # [ci, half, k(row-major), co]
    perm = [kh * KS + kw for (kh, kw) in TAPS_TOP]
    w_t = np.ascontiguousarray(w4[:, :, perm, :].astype(np.float16))
    b = np.ascontiguousarray(bias, dtype=np.float32)
    return blk, w_t, b


def kernel(x, weight, bias):
    from concourse.bass_utils import run_bass_kernel_spmd

    xb, w_t, b = _prep_inputs(x, weight, bias)
    nc = _get_nc()
    in_maps = [
        {"x": xb[i * N_PER : (i + 1) * N_PER], "w": w_t, "b": b}
        for i in range(N_CORES)
    ]
    res = run_bass_kernel_spmd(nc, in_maps, list(range(N_CORES)))
    y = np.concatenate(
        [
            res.results[i]["y"].reshape(N_PER, C_OUT, H, W).astype(np.float32)
            for i in range(N_CORES)
        ],
        axis=0,
    )
    return y


# revision 38
# speedup vs baseline: 1.0583x; 1.0583x over previous
"""Conv2d(128->256, 3x3, pad 1, stride 1) on 32x56x56 fp32, for 8 trn2 cores.

Strategy: data-parallel over batch N=32 -> 4 images/core. Per core an
implicit-GEMM conv: C_in=128 is the partition (contraction) dim; for each
(kh, kw) tap a [128ci x 128co] weight tile multiplies a shifted window of the
column-padded input image held in SBUF, accumulating into PSUM over the 9 taps.
Output rows are processed in chunks of 8 (free dim 8*56=448 <= 512 PSUM bank).
Matmuls run in float16 (fp16 keeps ~2.6e-4 rel err) with fp32 PSUM accumulate.

Layout details (all tuned from perfetto traces):
- Two SBUF copies of each input slice: copy A with the row interior at column
  1 (serves kw=0 and kw=2 taps) and copy B at column 2 (serves kw=1). This
  keeps every matmul rhs at an even fp16 element offset; odd offsets cost ~18
  extra PE cycles per matmul (SBUF word-split reads).
- No zero pad ROWS: boundary chunks instead shrink the kh taps that would
  read them (448 -> 392 free, PSUM sub-range), saving ~1.1us of PE work.
  Copy A keeps zero pad COLUMNS (elems 0 and 57 of each 58-elem row).
- PE warmup: dummy matmuls opened ASAP (wu memset on the gpsimd queue, which
  starts ~1us before vector) keep the PE busy while head DMAs land, so the
  HAM clock gate (opens ~6us after first PE activity, half clock until then)
  is already open when the real stream runs.
- Head DMAs: first x slice is only 9 rows (enough for chunk 0) and the tap-
  first weight block is a separate small DMA, so the first real matmul's
  dependencies land as early as possible.
- Outputs: full-half bulk DMAs for images 0..2 (12.5KB/partition
  descriptors), but CHUNK-wise for the whole last image so the final chunk's
  writeout is not queued behind a 1.6MB bulk transfer in the DMA FIFOs.
"""
import numpy as np
from contextlib import ExitStack

N_FULL, C_IN, H, W = 32, 128, 56, 56
C_OUT, KS = 256, 3
N_CORES = 8
N_PER = N_FULL // N_CORES          # 4 images per core
PIX = H * W                         # 3136
ROWS = 8                            # output rows per psum chunk
RC = H // ROWS                      # 7 chunks
NF = ROWS * W                       # 448 free elems per matmul
NARROW = (ROWS - 1) * W             # 392 free elems for boundary taps
SW = 58                             # padded row stride (56 + 2 border cols)

T_ROWS = 33                         # top tile: x rows 0..32 (chunks 0-3)
B_ROWS = 25                         # bottom tile: x rows 31..55 (chunks 4-6)
B0 = 31                             # first x row held in the bottom tile
XT_A = 9                            # first sub-DMA: x rows 0..8 (chunk 0)
BLKA = (T_ROWS + B_ROWS) * SW       # host-prepadded A block: 3364 elems

# tap order per chunk kind: (kh, kw) tuples. start=first, stop=last.
# narrow taps (boundary rows) must not be first or last.
TAPS_MID = [(0, 0), (0, 1), (0, 2), (1, 0), (1, 1), (1, 2), (2, 0), (2, 1), (2, 2)]
TAPS_TOP = [(1, 0), (1, 1), (1, 2), (0, 0), (0, 1), (0, 2), (2, 0), (2, 1), (2, 2)]
TAPS_BOT = [(0, 0), (0, 1), (0, 2), (2, 0), (2, 1), (2, 2), (1, 0), (1, 1), (1, 2)]

_CACHE = {}


def _build():
    import concourse.tile as tile
    from concourse import mybir, bacc

    f32 = mybir.dt.float32
    f16 = mybir.dt.float16

    nc = bacc.Bacc("TRN2", target_bir_lowering=False, debug=False)
    # host-prepadded A copy: per image 58 rows ([0..32] top, [31..55] bottom)
    # of [0, x_r(56), 0] -> matmul-ready for kw=0/2 taps straight from DMA;
    # only the kw=1 alignment copy (B, interior at col 2) is built on-device.
    x_d = nc.dram_tensor("x", [N_PER, C_IN, BLKA], f16, kind="ExternalInput").ap()
    # host-pretransposed: [ci, half, k, co_half] (half-major, contiguous per
    # half); within each half tap k=3 (the first tap chunk 0 runs) is stored
    # first: host order TAPS_TOP.
    w_d = nc.dram_tensor("w", [C_IN, 2, KS * KS, 128], f16, kind="ExternalInput").ap()
    b_d = nc.dram_tensor("b", [C_OUT], f32, kind="ExternalInput").ap()
    # fp16 output (upcast on host): halves output DMA bytes; ~5e-4 rel err
    y_d = nc.dram_tensor("y", [N_PER, C_OUT, PIX], f16, kind="ExternalOutput").ap()

    # host-order index of tap (kh, kw) inside a weight half (TAPS_TOP order)
    widx = {t: i for i, t in enumerate(TAPS_TOP)}

    with tile.TileContext(nc) as tc:
        with ExitStack() as ctx:
            wp = ctx.enter_context(tc.tile_pool(name="wp", bufs=1))
            # A blocks land whole per image, prefetched dependency-free at
            # kernel start: descriptors sit behind image-0's in the ring
            # FIFOs, and all input traffic completes before the first bulk
            # output DMA can starve it.
            xa = ctx.enter_context(tc.tile_pool(name="xa", bufs=3))
            xbt = ctx.enter_context(tc.tile_pool(name="xbt", bufs=2))
            xbb = ctx.enter_context(tc.tile_pool(name="xbb", bufs=2))
            pp = ctx.enter_context(tc.tile_pool(name="pp", bufs=6, space="PSUM"))
            op = ctx.enter_context(tc.tile_pool(name="op", bufs=2))

            # PE warmup: the clock ramps to full speed only after ~3-6us of
            # CONTINUOUS PE execution, and re-gates after ~1us idle. Keep the
            # PE busy with dummies from as early as possible until real work
            # is ready (~8.5us). The first dummies read the framework's
            # const-AP (materialized during init, no runtime memset dep) so
            # they start right at the post-init barrier on every core; then
            # wu-based dummies, shrinking toward the end for a fine handoff.
            c0 = nc.const_aps.tensor(0.0, [128, 1], f32)
            wups = pp.tile([128, NF], f32, tag="ps")
            for _ in range(10):
                nc.tensor.matmul(wups[0:1, 0:1], c0, c0, start=True, stop=True)
            wu = wp.tile([128, 224], f16)
            nc.gpsimd.memset(wu[:], 0.0)
            # overprovision the runway: a core whose inputs land late pays
            # ~1.5us for a PE idle (gap + clock re-gate), while an excess
            # dummy costs only ~45ns of displaced half-clock work
            for _ in range(8):
                nc.tensor.matmul(wups[:, 0:224], wu[:, 0:128], wu[:], start=True, stop=True)
            for _ in range(16):
                nc.tensor.matmul(wups[:, 0:112], wu[:, 0:128], wu[:, 0:112], start=True, stop=True)

            # Weight half 0 in two pieces on the ACT ring: chunk 0's first six
            # taps' weights (192KB) gate the first matmuls; the rest follows.
            w_r = wp.tile([C_IN, 2 * KS * KS * 128], f16)
            w_r4 = w_r[:].rearrange("p (h k co) -> p h k co", h=2, k=KS * KS)
            nc.scalar.dma_start(
                w_r4[:, 0, 0:6], w_d[:, 0, 0:6].rearrange("ci k co -> ci (k co)")
            )
            nc.scalar.dma_start(
                w_r4[:, 0, 6:9], w_d[:, 0, 6:9].rearrange("ci k co -> ci (k co)")
            )

            bias_sb = wp.tile([128, 2], f32)

            for n in range(N_PER):
                xat = xa.tile([C_IN, BLKA], f16)
                if n == 0:
                    # image 0's A block in three slices so chunk 0 unblocks
                    # as early as possible
                    nc.sync.dma_start(xat[:, 0 : XT_A * SW], x_d[n, :, 0 : XT_A * SW])
                    nc.sync.dma_start(
                        xat[:, XT_A * SW : T_ROWS * SW],
                        x_d[n, :, XT_A * SW : T_ROWS * SW],
                    )
                    nc.sync.dma_start(xat[:, T_ROWS * SW :], x_d[n, :, T_ROWS * SW :])
                    # weight half 1 + bias queue behind the critical pieces
                    nc.scalar.dma_start(
                        w_r4[:, 1], w_d[:, 1].rearrange("ci k co -> ci (k co)")
                    )
                    nc.scalar.dma_start(bias_sb[:], b_d.rearrange("(h p) -> p h", h=2))
                else:
                    nc.sync.dma_start(xat[:], x_d[n])

                xptA3 = xat[:, 0 : T_ROWS * SW].rearrange("p (a b) -> p a b", a=T_ROWS)
                xpbA3 = xat[:, T_ROWS * SW :].rearrange("p (a b) -> p a b", a=B_ROWS)

                # only the kw=1 alignment copy (B, interior at col 2) is made
                # on-device, on the DVE (Pool runs copies ~6.5x slower); the
                # bias-adds live on the ACT engine so a copy can never
                # head-of-line-block an ADD (which would back up PSUM and
                # stall the PE at image boundaries).
                xptB = xbt.tile([C_IN, T_ROWS * SW], f16)
                xptB3 = xptB[:].rearrange("p (a b) -> p a b", a=T_ROWS)
                if n == 0:
                    nc.vector.tensor_copy(
                        xptB3[:, 0:XT_A, 2 : 2 + W], xptA3[:, 0:XT_A, 1 : 1 + W]
                    )
                    nc.vector.tensor_copy(
                        xptB3[:, XT_A:, 2 : 2 + W], xptA3[:, XT_A:, 1 : 1 + W]
                    )
                else:
                    nc.vector.tensor_copy(xptB3[:, :, 2 : 2 + W], xptA3[:, :, 1 : 1 + W])
                xpbB = xbb.tile([C_IN, B_ROWS * SW], f16)
                xpbB3 = xpbB[:].rearrange("p (a b) -> p a b", a=B_ROWS)
                nc.vector.tensor_copy(xpbB3[:, :, 2 : 2 + W], xpbA3[:, :, 1 : 1 + W])

                out_sb = op.tile([128, 2 * PIX], f16)
                last_img = n == N_PER - 1
                for half in range(2):
                    # the very last half ends with 6-row + 2-row chunks so the
                    # post-final-matmul ADD+DMA tail is tiny
                    if last_img and half == 1:
                        chunks = [(i * ROWS, ROWS) for i in range(6)] + [(48, 6), (54, 2)]
                    else:
                        chunks = [(i * ROWS, ROWS) for i in range(RC)]
                    for ci, (r0c, nr) in enumerate(chunks):
                        nfc = nr * W
                        ps = pp.tile([128, NF], f32)
                        top_chunk = r0c == 0
                        bot_chunk = r0c + nr == H
                        taps = TAPS_TOP if top_chunk else (TAPS_BOT if bot_chunk else TAPS_MID)
                        for i, (kh, kw) in enumerate(taps):
                            lhsT = w_r4[:, half, widx[(kh, kw)], :]
                            # output rows r0c..r0c+nr-1 read x rows r0c+kh-1..
                            r0 = r0c + kh - 1
                            narrow_top = top_chunk and kh == 0     # skip out row 0
                            narrow_bot = bot_chunk and kh == 2     # skip out row 55
                            if r0c < 32:
                                A3, B3, base = xptA3, xptB3, 0
                            else:
                                A3, B3, base = xpbA3, xpbB3, B0
                            src3 = B3 if kw == 1 else A3
                            coff = 2 if kw == 1 else kw
                            if narrow_top:
                                rhs = src3[:, 0 : nr - 1, coff : coff + W]
                                dst = ps[:, W:nfc]
                            elif narrow_bot:
                                lr = r0 - base
                                rhs = src3[:, lr : lr + nr - 1, coff : coff + W]
                                dst = ps[:, 0 : nfc - W]
                            else:
                                lr = r0 - base
                                rhs = src3[:, lr : lr + nr, coff : coff + W]
                                dst = ps[:, 0:nfc]
                            nc.tensor.matmul(
                                dst, lhsT, rhs,
                                start=(i == 0), stop=(i == KS * KS - 1),
                            )
                        # psum -> sbuf with per-channel bias add (f32 -> f16)
                        # on the ACT engine: keeps the DVE free for pad copies
                        lo = half * PIX + r0c * W
                        nc.scalar.activation(
                            out_sb[:, lo : lo + nfc],
                            ps[:, 0:nfc],
                            mybir.ActivationFunctionType.Identity,
                            bias=bias_sb[:, half : half + 1],
                        )
                        if last_img:
                            # last image: stream output out as produced so
                            # nothing bulky queues ahead of the tail. Pair up
                            # 8-row chunks (fewer DMAs -> fewer semaphores to
                            # clear in teardown); alternate rings so
                            # consecutive writes drain in parallel.
                            paired = (
                                nr == ROWS and ci % 2 == 0
                                and ci + 1 < len(chunks) and chunks[ci + 1][1] == ROWS
                            )
                            if paired:
                                pass  # written together with the next chunk
                            else:
                                w0 = (r0c - ROWS) * W if (nr == ROWS and ci % 2 == 1) else r0c * W
                                eng = nc.sync if (half == 0) == (ci % 2 == 0) else nc.gpsimd
                                eng.dma_start(
                                    y_d[n, half * 128 : (half + 1) * 128, w0 : r0c * W + nfc],
                                    out_sb[:, half * PIX + w0 : lo + nfc],
                                )
                    if not last_img:
                        eng = nc.scalar if half == 0 else nc.gpsimd
                        eng.dma_start(
                            y_d[n, half * 128 : (half + 1) * 128, :],
                            out_sb[:, half * PIX : (half + 1) * PIX],
                        )
    nc.compile()
    return nc


def _get_nc():
    if "nc" not in _CACHE:
        _CACHE["nc"] = _build()
    return _CACHE["nc"]


def _prep_inputs(x, weight, bias):
    # host-prepadded A copy (fp16): per image 58 rows ([0..32] top then
    # [31..55] bottom) of [0, x_r(56), 0]
    xf = np.asarray(x, dtype=np.float32).astype(np.float16)
    blk = np.zeros((N_FULL, C_IN, T_ROWS + B_ROWS, SW), dtype=np.float16)
    blk[:, :, 0:T_ROWS, 1 : 1 + W] = xf[:, :, 0:T_ROWS, :]
    blk[:, :, T_ROWS:, 1 : 1 + W] = xf[:, :, B0 : B0 + B_ROWS, :]
    x = np.ascontiguousarray(blk.reshape(N_FULL, C_IN, BLKA))
    # [co, ci, kh, kw] -> [ci, half, k, co_half], half-major; taps within a
    # half stored in TAPS_TOP order so chunk-0's first weights DMA first.
    w4 = (
        np.transpose(np.asarray(weight, dtype=np.float32), (1, 2, 3, 0))
        .reshape(C_IN, KS * KS, 2, 128)
        .transpose(0, 2, 1, 3)
    )  # [ci, half, k(row-major), co]
    perm = [kh * KS + kw for (kh, kw) in TAPS_TOP]
    w_t = np.ascontiguousarray(w4[:, :, perm, :].astype(np.float16))
    b = np.ascontiguousarray(bias, dtype=np.float32)
    return x, w_t, b


def kernel(x, weight, bias):
    from concourse.bass_utils import run_bass_kernel_spmd

    x, w_t, b = _prep_inputs(x, weight, bias)
    nc = _get_nc()
    in_maps = [
        {"x": x[i * N_PER : (i + 1) * N_PER], "w": w_t, "b": b}
        for i in range(N_CORES)
    ]
    res = run_bass_kernel_spmd(nc, in_maps, list(range(N_CORES)))
    y = np.concatenate(
        [
            res.results[i]["y"].reshape(N_PER, C_OUT, H, W).astype(np.float32)
            for i in range(N_CORES)
        ],
        axis=0,
    )
    return y
